# revision 8
# baseline (speedup 1.0000x reference)
"""Multi-head attention (N=2, K=2048, DIN=1024, H=16, DOUT=64) on 8 TRN2 NeuronCores.

Sharding: data-parallel over batch N (cores 0-3 -> n=0, cores 4-7 -> n=1),
tensor-parallel over heads (4 heads per core). Each core computes its 4 heads'
attention plus the partial output projection over its head-feature slice of Wp;
the host sums the 4 partials per batch element and adds the output bias.

v3 restructure (from perfetto analysis of the v1 kernel):
  - steady-state pacing target is the ScalarE exp stream (2 x [128,1024] exps
    per kt tile = ~2.1us); everything else must hide under it.
  - PV/denominator matmuls trail the S/exp stage by TWO kt tiles, so the PE
    queue never in-order-blocks on the DVE mask-multiply of the current tile
    (v1 trailed by one and paid a ~500ns/kt three-engine round-trip).
  - denominator rows are M=32-replicated ones-matmuls (same cost as M=1 in
    the 4-way column-group pack); one in-place whole-bank PSUM reciprocal per
    block replaces v1's per-(head,qi) copy+reciprocal+K=1-broadcast-matmul
    chain (~19us PE + ~18us DVE). The normalize multiplies read the PSUM
    reciprocal directly (PSUM operands are exempt from the verifier's
    same-start-partition rule for tensor_tensor).
  - the two per-kt mask multiplies fuse into one [128, 2, 1024] DVE op with
    the mask tile broadcast across the head dim.
  - projections and the output projection are not phases: they are injected
    tile-by-tile into the attention blocks' PE slack. All DMA (inputs, masks)
    is emitted in strict consumption order - per-queue FIFO plus shared HBM
    bandwidth means anything emitted early delays everything behind it.
  - PSUM map (8 banks): S double-buffer 2x[128,1024] (4), PV accumulators
    2x[128,512] (2), denominator [128,512] (1), aux for injected tiles (1).
"""

import numpy as np
import ml_dtypes

import concourse.bass as bass
import concourse.mybir as mybir
from concourse import bacc
from concourse.tile import TileContext

P = 128
SEQ = 2048
DIN = 1024
DOUT = 64
H = 16
N = 2
HPC = 4  # heads per core
NPAIR = 2  # head pairs per core
KSUB = DIN // P  # 8 contraction subtiles for projections
NKT = SEQ // P  # 16 seq_k tiles of 128
BF = mybir.dt.bfloat16
F32 = mybir.dt.float32
BF_NP = ml_dtypes.bfloat16

_NC_CACHE = None


def build_bass():
    nc = bacc.Bacc()

    xq_d = nc.declare_dram_parameter("xqT", [DIN, SEQ], BF, isOutput=False)
    xk_d = nc.declare_dram_parameter("xkT", [DIN, SEQ], BF, isOutput=False)
    xv_d = nc.declare_dram_parameter("xvT", [DIN, SEQ], BF, isOutput=False)
    mk_d = nc.declare_dram_parameter("maskT", [SEQ, SEQ], BF, isOutput=False)
    wq_d = nc.declare_dram_parameter("wq", [DIN, HPC * DOUT], BF, isOutput=False)
    wk_d = nc.declare_dram_parameter("wk", [DIN, HPC * DOUT], BF, isOutput=False)
    wv_d = nc.declare_dram_parameter("wv", [DIN, HPC * DOUT], BF, isOutput=False)
    wp_d = nc.declare_dram_parameter("wp", [HPC * DOUT, DIN], BF, isOutput=False)
    bqp_d = nc.declare_dram_parameter("bqp", [P, NPAIR], F32, isOutput=False)
    bkp_d = nc.declare_dram_parameter("bkp", [P, NPAIR], F32, isOutput=False)
    bvr_d = nc.declare_dram_parameter("bvr", [P, HPC * DOUT], F32, isOutput=False)
    out_d = nc.declare_dram_parameter("out", [SEQ, DIN], F32, isOutput=True)

    ADD = mybir.AluOpType.add
    MUL = mybir.AluOpType.mult
    EXP = mybir.ActivationFunctionType.Exp

    with TileContext(nc) as tc:
        with (
            tc.tile_pool(name="const", bufs=1) as const,
            tc.tile_pool(name="xin", bufs=1) as xin,
            tc.tile_pool(name="proj", bufs=1) as proj,
            tc.tile_pool(name="maskp", bufs=7) as maskp,
            tc.tile_pool(name="ptp", bufs=2) as ptp,
            tc.tile_pool(name="epi", bufs=2) as epi,
            tc.tile_pool(name="ps_s", bufs=2, space="PSUM") as ps_s,
            tc.tile_pool(name="ps_pv", bufs=2, space="PSUM") as ps_pv,
            tc.tile_pool(name="ps_dn", bufs=1, space="PSUM") as ps_dn,
            tc.tile_pool(name="ps_aux", bufs=1, space="PSUM") as ps_aux,
        ):
            # ---- weight/bias constants (small, land first) ------------------
            wq_sb = const.tile([P, KSUB, HPC * DOUT], BF)
            nc.sync.dma_start(wq_sb[:], wq_d.rearrange("(o p) m -> p o m", p=P))
            wk_sb = const.tile([P, KSUB, HPC * DOUT], BF)
            nc.sync.dma_start(wk_sb[:], wk_d.rearrange("(o p) m -> p o m", p=P))
            wv_sb = const.tile([P, KSUB, HPC * DOUT], BF)
            nc.sync.dma_start(wv_sb[:], wv_d.rearrange("(o p) m -> p o m", p=P))
            wp_sb = const.tile([P, NPAIR, DIN], BF)
            nc.sync.dma_start(wp_sb[:], wp_d.rearrange("(o p) n -> p o n", p=P))
            bqp_sb = const.tile([P, NPAIR], F32)
            nc.sync.dma_start(bqp_sb[:], bqp_d[:])
            bkp_sb = const.tile([P, NPAIR], F32)
            nc.sync.dma_start(bkp_sb[:], bkp_d[:])
            bvr_sb = const.tile([P, HPC * DOUT], F32)
            nc.sync.dma_start(bvr_sb[:], bvr_d[:])
            # M=32-replicated ones: lhsT for the denominator row-sum matmuls
            ones32_sb = const.tile([P, 32], BF)
            nc.vector.memset(ones32_sb[:], 1.0)

            warm_rhs = const.tile([P, 512], BF)
            nc.vector.memset(warm_rhs[:], 1.0)

            # ---- input tiles; DMA emitted in strict consumption order --------
            xq_sb = xin.tile([P, KSUB, SEQ], BF)
            xk_sb = xin.tile([P, KSUB, SEQ], BF)
            xv_sb = xin.tile([P, KSUB, SEQ], BF)
            xq_r = xq_d.rearrange("(o p) s -> p o s", p=P)
            xk_r = xk_d.rearrange("(o p) s -> p o s", p=P)
            xv_r = xv_d.rearrange("(o p) s -> p o s", p=P)

            def dma_x(x_sb, x_r, lo, hi):
                def th():
                    for o in range(KSUB):
                        nc.sync.dma_start(x_sb[:, o, lo:hi], x_r[:, o, lo:hi])

                return th

            # pre-A bulk: what the prefix + first attention tiles need
            dma_x(xq_sb, xq_r, 0, 1024)()  # q-proj qt0,1
            dma_x(xk_sb, xk_r, 0, 512)()  # k-proj qt0
            dma_x(xv_sb, xv_r, 0, 512)()  # v st0-3

            # mask tiles for block A, kt0-4 (rest emitted inside block A)
            def mask_dma(kt, qh):
                mt = maskp.tile([P, 1024], BF, tag="mt", name="mt")
                nc.sync.dma_start(
                    mt[:], mk_d[kt * P : (kt + 1) * P, qh * 1024 : (qh + 1) * 1024]
                )
                return mt

            premasks_a = {}
            for k in range(3):
                premasks_a[k] = mask_dma(k, 0)
            dma_x(xk_sb, xk_r, 512, 1024)()  # k-proj qt1 (inject A kt0)
            for k in (3, 4):
                premasks_a[k] = mask_dma(k, 0)

            # remaining bulk, emitted inside block A in consumption order
            dma_feed_a = {
                0: [dma_x(xk_sb, xk_r, 1024, 1536)],
                1: [dma_x(xv_sb, xv_r, 512, 1024)],
                3: [dma_x(xk_sb, xk_r, 1536, 2048)],
                4: [dma_x(xq_sb, xq_r, 1024, 1536)],
                5: [dma_x(xq_sb, xq_r, 1536, 2048)],
                6: [dma_x(xv_sb, xv_r, 1024, 1536)],
                8: [dma_x(xv_sb, xv_r, 1536, 2048)],
            }

            # ---- persistent intermediates ------------------------------------
            qhT = proj.tile([P, NPAIR, SEQ], BF)  # [2-head dout, pair, seq]
            khT = proj.tile([P, NPAIR, SEQ], BF)
            vsb = proj.tile([P, NKT, HPC * DOUT], BF)  # v natural [seq, dout]
            ynT = proj.tile([P, NPAIR, SEQ], BF)  # normalized y.T
            vsb4 = vsb.rearrange("p k (h c) -> p k h c", c=DOUT)

            # PE warmup: bring the HAM clock gate to 8/8 while the first input
            # chunks are still in flight; result never read.
            warm_ps = ps_aux.tile([P, 512], F32, tag="aux", name="warm_ps")
            for _ in range(32):
                nc.tensor.matmul(
                    warm_ps[:, 0:256],
                    warm_rhs[:, 0:P],
                    warm_rhs[:, 0:256],
                    start=True,
                    stop=True,
                    skip_group_check=True,
                )

            # ---- projection tile emitters ------------------------------------
            def qk_tile(which, pair, qt, pool, tag):
                w_sb, x_sb, b_sb, o_sb = {
                    "q": (wq_sb, xq_sb, bqp_sb, qhT),
                    "k": (wk_sb, xk_sb, bkp_sb, khT),
                }[which]
                pps = pool.tile([P, 512], F32, tag=tag, name=f"pps_{which}{pair}{qt}")
                for o in range(KSUB):
                    nc.tensor.matmul(
                        pps[:],
                        w_sb[:, o, pair * P : (pair + 1) * P],
                        x_sb[:, o, qt * 512 : (qt + 1) * 512],
                        start=(o == 0),
                        stop=(o == KSUB - 1),
                    )
                nc.vector.tensor_tensor(
                    o_sb[:, pair, qt * 512 : (qt + 1) * 512],
                    pps[:],
                    b_sb[:, pair : pair + 1].to_broadcast((P, 512)),
                    ADD,
                )

            def v_tile(st, pool, tag):
                vps = pool.tile([P, HPC * DOUT], F32, tag=tag, name=f"vps{st}")
                for o in range(KSUB):
                    nc.tensor.matmul(
                        vps[:],
                        xv_sb[:, o, st * P : (st + 1) * P],
                        wv_sb[:, o, :],
                        start=(o == 0),
                        stop=(o == KSUB - 1),
                    )
                nc.vector.tensor_tensor(
                    vsb4[:, st, :, :],
                    vps.rearrange("p (h c) -> p h c", c=DOUT),
                    bvr_sb.rearrange("p (h c) -> p h c", c=DOUT),
                    ADD,
                )

            # ---- output projection piece (partial over this core's 4 heads) --
            def c_piece(st, ntile, pool, tag, drain_eng):
                cps = pool.tile([P, 512], F32, tag=tag, name=f"cps{st}_{ntile}")
                for pair in range(NPAIR):
                    nc.tensor.matmul(
                        cps[:],
                        ynT[:, pair, st * P : (st + 1) * P],
                        wp_sb[:, pair, ntile * 512 : (ntile + 1) * 512],
                        start=(pair == 0),
                        stop=(pair == NPAIR - 1),
                    )
                ost = epi.tile([P, 512], F32, tag="ost", name="ost", bufs=4)
                if drain_eng == "scalar":
                    nc.scalar.copy(ost[:], cps[:])
                else:
                    nc.vector.tensor_copy(ost[:], cps[:])
                nc.sync.dma_start(
                    out_d[st * P : (st + 1) * P, ntile * 512 : (ntile + 1) * 512],
                    ost[:],
                )

            # ---- deferred epilogue: normalize a finished block's PV ----------
            # pending entries: (pair, qh, pv_sbs, dn_psum) where dn_psum holds
            # the in-place reciprocal of the denominators (PSUM operand, so the
            # mixed start-partition multiply passes the BIR verifier).
            pending = []

            def flush_qi(ent, qi):
                pair, qh, pv_sbs, dnr = ent
                q0 = (qh * 2 + qi) * 512
                for h2 in range(2):
                    r = 32 * qi + 64 * h2
                    for j in range(2):
                        rows = slice(h2 * 64 + 32 * j, h2 * 64 + 32 * j + 32)
                        nc.vector.tensor_tensor(
                            ynT[rows, pair, q0 : q0 + 512],
                            pv_sbs[qi][rows, :],
                            dnr[r : r + 32, :],
                            MUL,
                        )

            # ---- attention block ----------------------------------------------
            def attn_block(pair, qh, inject, dma_feed=None, premasks=None):
                dma_feed = dma_feed or {}
                pvs = []
                for i in range(2):
                    pv = ps_pv.tile([P, 512], F32, tag="pv", name=f"pv{i}")
                    nc.vector.memset(pv[:], 0.0)
                    pvs.append(pv)
                dn = ps_dn.tile([P, 512], F32, tag="dn", name="dn")

                def pv_dn_mms(kt, ptm):
                    for qi in range(2):
                        for h2 in range(2):
                            nc.tensor.matmul(
                                pvs[qi][h2 * DOUT : (h2 + 1) * DOUT, :],
                                vsb4[:, kt, pair * 2 + h2, :],
                                ptm[:, h2, qi * 512 : (qi + 1) * 512],
                                start=False,
                                stop=(kt == NKT - 1),
                                tile_position=(0, h2 * DOUT),
                                skip_group_check=True,
                            )
                    for qi in range(2):
                        for h2 in range(2):
                            row = 32 * qi + 64 * h2
                            nc.tensor.matmul(
                                dn[row : row + 32, :],
                                ones32_sb[:],
                                ptm[:, h2, qi * 512 : (qi + 1) * 512],
                                start=False,
                                stop=(kt == NKT - 1),
                                tile_position=(0, row),
                                skip_group_check=True,
                            )

                masks = dict(premasks) if premasks else {}
                if premasks is None:
                    for k in range(3):
                        masks[k] = mask_dma(k, qh)
                ahead = 5 if premasks else 3

                ptms = {}
                for kt in range(NKT):
                    for th in dma_feed.get(kt, ()):
                        th()
                    # previous block's epilogue, spread over early kts
                    # (qi0 first so phaseC pieces over low q unblock sooner)
                    if pending and kt == 1:
                        flush_qi(pending[0], 0)
                    if pending and kt == 2:
                        flush_qi(pending[0], 1)
                    if kt == 2:
                        # after the old denominators' last read, before the
                        # first denominator matmul of this block
                        nc.vector.memset(dn[:], 0.0)
                    # trailing PV/dn: ready two tiles ago, fills the window
                    # while ScalarE still reads the previous S slots
                    if kt >= 2:
                        pv_dn_mms(kt - 2, ptms.pop(kt - 2))
                    for th in inject.get(kt, ()):
                        th()
                    if kt + ahead < NKT and (kt + ahead) not in masks:
                        masks[kt + ahead] = mask_dma(kt + ahead, qh)
                    sps = []
                    for h2 in range(2):
                        sp = ps_s.tile([P, 1024], F32, tag="s", name=f"sps{h2}")
                        sps.append(sp)
                    for qi in range(2):
                        for h2 in range(2):
                            q0 = (qh * 2 + qi) * 512
                            nc.tensor.matmul(
                                sps[h2][:, qi * 512 : (qi + 1) * 512],
                                khT[h2 * DOUT : (h2 + 1) * DOUT, pair, kt * P : (kt + 1) * P],
                                qhT[h2 * DOUT : (h2 + 1) * DOUT, pair, q0 : q0 + 512],
                                start=True,
                                stop=True,
                            )
                    pt = ptp.tile([P, 2, 1024], BF, tag="pt", name="pt", bufs=3)
                    for h2 in range(2):
                        nc.scalar.activation(pt[:, h2, :], sps[h2][:], EXP, scale=0.125)
                    mt = masks.pop(kt)
                    ptm = ptp.tile([P, 2, 1024], BF, tag="ptm", name="ptm", bufs=3)
                    nc.vector.tensor_tensor(
                        ptm[:],
                        pt[:],
                        mt.rearrange("p (o n) -> p o n", o=1).to_broadcast((P, 2, 1024)),
                        MUL,
                    )
                    ptms[kt] = ptm

                if pending:
                    pending.pop(0)
                # trailing tail: last two kt tiles
                for kt in (NKT - 2, NKT - 1):
                    pv_dn_mms(kt, ptms.pop(kt))
                # drain PV accumulators; in-place reciprocal of the denominators
                pv_sbs = []
                for qi in range(2):
                    pv_sb = epi.tile([P, 512], F32, tag="pvsb", name="pv_sb", bufs=4)
                    nc.vector.tensor_copy(pv_sb[:], pvs[qi][:])
                    pv_sbs.append(pv_sb)
                nc.vector.reciprocal_approx_fast(dn[:], dn[:])
                pending.append((pair, qh, pv_sbs, dn))

            # ---- prefix: minimal projections to unblock block A ---------------
            qk_tile("q", 0, 0, ps_s, "s")
            qk_tile("q", 0, 1, ps_s, "s")
            qk_tile("k", 0, 0, ps_pv, "pv")
            v_tile(0, ps_pv, "pv")
            v_tile(1, ps_dn, "dn")

            # ---- block A: (pair0, qh0); inject v st2-15, k-p0 qt1-3, q-p0 qt2-3
            inj_a = {kt: [] for kt in range(NKT)}
            for kt in range(1, 15):  # v st2..15 at kt 1..14 (kt0 stays clean)
                st = kt + 1
                inj_a[kt].append(lambda st=st: v_tile(st, ps_aux, "aux"))
            inj_a[1].append(lambda: qk_tile("k", 0, 1, ps_aux, "aux"))
            inj_a[3].append(lambda: qk_tile("k", 0, 2, ps_aux, "aux"))
            inj_a[7].append(lambda: qk_tile("k", 0, 3, ps_aux, "aux"))
            inj_a[9].append(lambda: qk_tile("q", 0, 2, ps_aux, "aux"))
            inj_a[11].append(lambda: qk_tile("q", 0, 3, ps_aux, "aux"))
            attn_block(0, 0, inj_a, dma_feed=dma_feed_a, premasks=premasks_a)

            # ---- block B: (pair0, qh1); inject first half of pair-1 q/k proj --
            inj_b = {
                1: [lambda: qk_tile("q", 1, 0, ps_aux, "aux")],
                4: [lambda: qk_tile("q", 1, 1, ps_aux, "aux")],
                7: [lambda: qk_tile("k", 1, 0, ps_aux, "aux")],
                10: [lambda: qk_tile("k", 1, 1, ps_aux, "aux")],
            }
            attn_block(0, 1, inj_b)

            # ---- block C: (pair1, qh0); inject rest of pair-1 proj ------------
            inj_c = {
                1: [lambda: qk_tile("k", 1, 2, ps_aux, "aux")],
                5: [lambda: qk_tile("k", 1, 3, ps_aux, "aux")],
                8: [lambda: qk_tile("q", 1, 2, ps_aux, "aux")],
                10: [lambda: qk_tile("q", 1, 3, ps_aux, "aux")],
            }
            attn_block(1, 0, inj_c)

            # ---- block D: (pair1, qh1); inject phaseC st0-7 -------------------
            # C's epilogue flushes at kt1 (qi0: q<512) and kt2 (qi1), so pieces
            # over q[0,512) can start at kt3 and the rest from kt4.
            pieces = [(st, nt) for st in range(8) for nt in range(2)]
            sched = [(3, 1), (4, 1), (5, 1), (6, 1), (7, 2), (8, 1), (9, 2),
                     (10, 1), (11, 2), (12, 1), (13, 1), (14, 1), (15, 1)]
            inj_d = {}
            pi = 0
            for kt, cnt in sched:
                lst = []
                for j in range(cnt):
                    st, ntile = pieces[pi]
                    eng = "vector" if j == 0 else "scalar"
                    lst.append(
                        lambda st=st, ntile=ntile, eng=eng: c_piece(
                            st, ntile, ps_aux, "aux", eng
                        )
                    )
                    pi += 1
                inj_d[kt] = lst
            assert pi == 16
            attn_block(1, 1, inj_d)

            # ---- tail: flush D's epilogue, then phaseC st8-15 ------------------
            ent = pending.pop(0)
            flush_qi(ent, 0)
            tail_pools = [(ps_aux, "aux"), (ps_pv, "pv"), (ps_s, "s")]
            tp_i = 0
            for st in (8, 9, 10, 11):
                for ntile in range(2):
                    pool, tag = tail_pools[tp_i % 3]
                    eng = "scalar" if tp_i % 2 == 0 else "vector"
                    c_piece(st, ntile, pool, tag, eng)
                    tp_i += 1
                if st == 8:
                    flush_qi(ent, 1)
            for st in (12, 13, 14, 15):
                for ntile in range(2):
                    pool, tag = tail_pools[tp_i % 3]
                    eng = "scalar" if tp_i % 2 == 0 else "vector"
                    c_piece(st, ntile, pool, tag, eng)
                    tp_i += 1

    nc.finalize()
    return nc


def make_in_maps(query, key, value, mask, Wq, bq, Wk, bk, Wv, bv, Wp, bp):
    """Shard + pre-layout the full inputs into 8 per-core input dicts."""
    in_maps = []
    for c in range(8):
        n = c // 4
        h0 = HPC * (c % 4)
        hs = slice(h0, h0 + HPC)

        def t_bf(x):  # [SEQ, DIN] -> contiguous [DIN, SEQ] bf16
            return np.ascontiguousarray(x.T).astype(BF_NP)

        # (H', DIN, DOUT) -> (DIN, H'*DOUT), head-major columns
        def w_bf(W):
            return np.ascontiguousarray(
                W[hs].transpose(1, 0, 2).reshape(DIN, HPC * DOUT)
            ).astype(BF_NP)

        # per-pair per-partition bias: [128, 2], col p = concat of heads (2p, 2p+1)
        def b_pair(b):
            return np.ascontiguousarray(b[hs].reshape(NPAIR, P).T).astype(np.float32)

        in_maps.append(
            {
                "xqT": t_bf(query[n]),
                "xkT": t_bf(key[n]),
                "xvT": t_bf(value[n]),
                "maskT": np.ascontiguousarray((~mask[n]).T).astype(BF_NP),
                "wq": w_bf(Wq),
                "wk": w_bf(Wk),
                "wv": w_bf(Wv),
                "wp": np.ascontiguousarray(
                    Wp[h0 * DOUT : (h0 + HPC) * DOUT, :]
                ).astype(BF_NP),
                "bqp": b_pair(bq),
                "bkp": b_pair(bk),
                "bvr": np.ascontiguousarray(
                    np.tile(bv[hs].reshape(1, HPC * DOUT), (P, 1))
                ).astype(np.float32),
            }
        )
    return in_maps


def kernel(**inputs):
    global _NC_CACHE
    from concourse.bass_utils import run_bass_kernel_spmd

    if _NC_CACHE is None:
        _NC_CACHE = build_bass()
    nc = _NC_CACHE

    in_maps = make_in_maps(**inputs)
    res = run_bass_kernel_spmd(nc, in_maps, core_ids=list(range(8))).results
    parts = [res[c]["out"].astype(np.float32) for c in range(8)]
    bp = inputs["bp"]
    out = np.stack(
        [
            parts[0] + parts[1] + parts[2] + parts[3] + bp[None, :],
            parts[4] + parts[5] + parts[6] + parts[7] + bp[None, :],
        ]
    )
    return out.astype(np.float32)


# revision 11
# speedup vs baseline: 1.1815x; 1.1815x over previous
"""Multi-head attention (N=2, K=2048, DIN=1024, H=16, DOUT=64) on 8 TRN2 NeuronCores.

Sharding: data-parallel over batch N (cores 0-3 -> n=0, cores 4-7 -> n=1),
tensor-parallel over heads (4 heads per core). Each core computes its 4 heads'
attention plus the partial output projection over its head-feature slice of Wp;
the host sums the 4 partials per batch element and adds the output bias.

v3 restructure (from perfetto analysis of the v1 kernel):
  - steady-state pacing target is the ScalarE exp stream (2 x [128,1024] exps
    per kt tile = ~2.1us); everything else must hide under it.
  - PV/denominator matmuls trail the S/exp stage by TWO kt tiles, so the PE
    queue never in-order-blocks on the DVE mask-multiply of the current tile
    (v1 trailed by one and paid a ~500ns/kt three-engine round-trip).
  - denominator rows are M=32-replicated ones-matmuls (same cost as M=1 in
    the 4-way column-group pack); one in-place whole-bank PSUM reciprocal per
    block replaces v1's per-(head,qi) copy+reciprocal+K=1-broadcast-matmul
    chain (~19us PE + ~18us DVE). The normalize multiplies read the PSUM
    reciprocal directly (PSUM operands are exempt from the verifier's
    same-start-partition rule for tensor_tensor).
  - the two per-kt mask multiplies fuse into one [128, 2, 1024] DVE op with
    the mask tile broadcast across the head dim.
  - projections and the output projection are not phases: they are injected
    tile-by-tile into the attention blocks' PE slack. All DMA (inputs, masks)
    is emitted in strict consumption order - per-queue FIFO plus shared HBM
    bandwidth means anything emitted early delays everything behind it.
  - PSUM map (8 banks): S double-buffer 2x[128,1024] (4), PV accumulators
    2x[128,512] (2), denominator [128,512] (1), aux for injected tiles (1).
"""

import numpy as np
import ml_dtypes

import concourse.bass as bass
import concourse.mybir as mybir
from concourse import bacc
from concourse.tile import TileContext

P = 128
SEQ = 2048
DIN = 1024
DOUT = 64
H = 16
N = 2
HPC = 4  # heads per core
NPAIR = 2  # head pairs per core
KSUB = DIN // P  # 8 contraction subtiles for projections
NKT = SEQ // P  # 16 seq_k tiles of 128
BF = mybir.dt.bfloat16
F32 = mybir.dt.float32
BF_NP = ml_dtypes.bfloat16

_NC_CACHE = None


def build_bass():
    nc = bacc.Bacc()

    xq_d = nc.declare_dram_parameter("xqT", [DIN, SEQ], BF, isOutput=False)
    xk_d = nc.declare_dram_parameter("xkT", [DIN, SEQ], BF, isOutput=False)
    xv_d = nc.declare_dram_parameter("xvT", [DIN, SEQ], BF, isOutput=False)
    mk_d = nc.declare_dram_parameter("maskT", [SEQ, SEQ], BF, isOutput=False)
    wq_d = nc.declare_dram_parameter("wq", [DIN, HPC * DOUT], BF, isOutput=False)
    wk_d = nc.declare_dram_parameter("wk", [DIN, HPC * DOUT], BF, isOutput=False)
    wv_d = nc.declare_dram_parameter("wv", [DIN, HPC * DOUT], BF, isOutput=False)
    wp_d = nc.declare_dram_parameter("wp", [HPC * DOUT, DIN], BF, isOutput=False)
    bqp_d = nc.declare_dram_parameter("bqp", [P, NPAIR], F32, isOutput=False)
    bkp_d = nc.declare_dram_parameter("bkp", [P, NPAIR], F32, isOutput=False)
    bvr_d = nc.declare_dram_parameter("bvr", [P, HPC * DOUT], F32, isOutput=False)
    out_d = nc.declare_dram_parameter("out", [SEQ, DIN], BF, isOutput=True)

    ADD = mybir.AluOpType.add
    MUL = mybir.AluOpType.mult
    EXP = mybir.ActivationFunctionType.Exp

    with TileContext(nc) as tc:
        with (
            tc.tile_pool(name="const", bufs=1) as const,
            tc.tile_pool(name="xin", bufs=1) as xin,
            tc.tile_pool(name="proj", bufs=1) as proj,
            tc.tile_pool(name="maskp", bufs=7) as maskp,
            tc.tile_pool(name="ptp", bufs=2) as ptp,
            tc.tile_pool(name="epi", bufs=2) as epi,
            tc.tile_pool(name="ps_s", bufs=2, space="PSUM") as ps_s,
            tc.tile_pool(name="ps_pv", bufs=2, space="PSUM") as ps_pv,
            tc.tile_pool(name="ps_dn", bufs=1, space="PSUM") as ps_dn,
            tc.tile_pool(name="ps_aux", bufs=1, space="PSUM") as ps_aux,
        ):
            # ---- weight/bias constants (small, land first) ------------------
            wq_sb = const.tile([P, KSUB, HPC * DOUT], BF)
            nc.sync.dma_start(wq_sb[:], wq_d.rearrange("(o p) m -> p o m", p=P))
            wk_sb = const.tile([P, KSUB, HPC * DOUT], BF)
            nc.sync.dma_start(wk_sb[:], wk_d.rearrange("(o p) m -> p o m", p=P))
            wv_sb = const.tile([P, KSUB, HPC * DOUT], BF)
            nc.sync.dma_start(wv_sb[:], wv_d.rearrange("(o p) m -> p o m", p=P))
            wp_sb = const.tile([P, NPAIR, DIN], BF)
            nc.sync.dma_start(wp_sb[:], wp_d.rearrange("(o p) n -> p o n", p=P))
            bqp_sb = const.tile([P, NPAIR], F32)
            nc.sync.dma_start(bqp_sb[:], bqp_d[:])
            bkp_sb = const.tile([P, NPAIR], F32)
            nc.sync.dma_start(bkp_sb[:], bkp_d[:])
            bvr_sb = const.tile([P, HPC * DOUT], F32)
            nc.sync.dma_start(bvr_sb[:], bvr_d[:])
            # M=32-replicated ones: lhsT for the denominator row-sum matmuls
            ones32_sb = const.tile([P, 32], BF)
            nc.vector.memset(ones32_sb[:], 1.0)

            warm_rhs = const.tile([P, 512], BF)
            nc.vector.memset(warm_rhs[:], 1.0)

            # ---- input tiles; DMA emitted in strict consumption order --------
            xq_sb = xin.tile([P, KSUB, SEQ], BF)
            xk_sb = xin.tile([P, KSUB, SEQ], BF)
            xv_sb = xin.tile([P, KSUB, SEQ], BF)
            xq_r = xq_d.rearrange("(o p) s -> p o s", p=P)
            xk_r = xk_d.rearrange("(o p) s -> p o s", p=P)
            xv_r = xv_d.rearrange("(o p) s -> p o s", p=P)

            def dma_x(x_sb, x_r, lo, hi):
                def th():
                    for o in range(KSUB):
                        nc.sync.dma_start(x_sb[:, o, lo:hi], x_r[:, o, lo:hi])

                return th

            # pre-A bulk: what the prefix + first attention tiles need
            dma_x(xq_sb, xq_r, 0, 1024)()  # q-proj qt0,1
            dma_x(xk_sb, xk_r, 0, 512)()  # k-proj qt0
            dma_x(xv_sb, xv_r, 0, 512)()  # v st0-3

            # mask tiles for block A, kt0-4 (rest emitted inside block A)
            def mask_dma(kt, qh):
                mt = maskp.tile([P, 1024], BF, tag="mt", name="mt")
                nc.sync.dma_start(
                    mt[:], mk_d[kt * P : (kt + 1) * P, qh * 1024 : (qh + 1) * 1024]
                )
                return mt

            premasks_a = {}
            for k in range(3):
                premasks_a[k] = mask_dma(k, 0)
            dma_x(xk_sb, xk_r, 512, 1024)()  # k-proj qt1 (inject A kt0)
            for k in (3, 4):
                premasks_a[k] = mask_dma(k, 0)

            # remaining bulk, emitted inside block A in consumption order
            dma_feed_a = {
                0: [dma_x(xk_sb, xk_r, 1024, 1536)],
                1: [dma_x(xv_sb, xv_r, 512, 1024)],
                3: [dma_x(xk_sb, xk_r, 1536, 2048)],
                4: [dma_x(xq_sb, xq_r, 1024, 1536)],
                5: [dma_x(xq_sb, xq_r, 1536, 2048)],
                6: [dma_x(xv_sb, xv_r, 1024, 1536)],
                8: [dma_x(xv_sb, xv_r, 1536, 2048)],
            }

            # ---- persistent intermediates ------------------------------------
            qhT = proj.tile([P, NPAIR, SEQ], BF)  # [2-head dout, pair, seq]
            khT = proj.tile([P, NPAIR, SEQ], BF)
            vsb = proj.tile([P, NKT, HPC * DOUT], BF)  # v natural [seq, dout]
            ynT = proj.tile([P, NPAIR, SEQ], BF)  # normalized y.T
            vsb4 = vsb.rearrange("p k (h c) -> p k h c", c=DOUT)

            # PE warmup: bring the HAM clock gate to 8/8 while the first input
            # chunks are still in flight; result never read.
            warm_ps = ps_aux.tile([P, 512], F32, tag="aux", name="warm_ps")
            for _ in range(32):
                nc.tensor.matmul(
                    warm_ps[:, 0:256],
                    warm_rhs[:, 0:P],
                    warm_rhs[:, 0:256],
                    start=True,
                    stop=True,
                    skip_group_check=True,
                )

            # ---- projection tile emitters ------------------------------------
            def qk_tile(which, pair, qt, pool, tag):
                w_sb, x_sb, b_sb, o_sb = {
                    "q": (wq_sb, xq_sb, bqp_sb, qhT),
                    "k": (wk_sb, xk_sb, bkp_sb, khT),
                }[which]
                pps = pool.tile([P, 512], F32, tag=tag, name=f"pps_{which}{pair}{qt}")
                for o in range(KSUB):
                    nc.tensor.matmul(
                        pps[:],
                        w_sb[:, o, pair * P : (pair + 1) * P],
                        x_sb[:, o, qt * 512 : (qt + 1) * 512],
                        start=(o == 0),
                        stop=(o == KSUB - 1),
                    )
                nc.vector.tensor_tensor(
                    o_sb[:, pair, qt * 512 : (qt + 1) * 512],
                    pps[:],
                    b_sb[:, pair : pair + 1].to_broadcast((P, 512)),
                    ADD,
                )

            def v_tile(st, pool, tag):
                vps = pool.tile([P, HPC * DOUT], F32, tag=tag, name=f"vps{st}")
                for o in range(KSUB):
                    nc.tensor.matmul(
                        vps[:],
                        xv_sb[:, o, st * P : (st + 1) * P],
                        wv_sb[:, o, :],
                        start=(o == 0),
                        stop=(o == KSUB - 1),
                    )
                nc.vector.tensor_tensor(
                    vsb4[:, st, :, :],
                    vps.rearrange("p (h c) -> p h c", c=DOUT),
                    bvr_sb.rearrange("p (h c) -> p h c", c=DOUT),
                    ADD,
                )

            # ---- output projection piece (partial over this core's 4 heads) --
            def c_piece(st, ntile, pool, tag, drain_eng):
                cps = pool.tile([P, 512], F32, tag=tag, name=f"cps{st}_{ntile}")
                for pair in range(NPAIR):
                    nc.tensor.matmul(
                        cps[:],
                        ynT[:, pair, st * P : (st + 1) * P],
                        wp_sb[:, pair, ntile * 512 : (ntile + 1) * 512],
                        start=(pair == 0),
                        stop=(pair == NPAIR - 1),
                    )
                ost = epi.tile([P, 512], BF, tag="ost", name="ost", bufs=4)
                if drain_eng == "scalar":
                    nc.scalar.copy(ost[:], cps[:])
                else:
                    nc.vector.tensor_copy(ost[:], cps[:])
                nc.sync.dma_start(
                    out_d[st * P : (st + 1) * P, ntile * 512 : (ntile + 1) * 512],
                    ost[:],
                )

            # ---- deferred epilogue: normalize a finished block's PV ----------
            # pending entries: (pair, qh, pv_sbs, dn_psum) where dn_psum holds
            # the in-place reciprocal of the denominators (PSUM operand, so the
            # mixed start-partition multiply passes the BIR verifier).
            pending = []

            def flush_qi(ent, qi):
                pair, qh, pv_sbs, dnr = ent
                q0 = (qh * 2 + qi) * 512
                for h2 in range(2):
                    r = 32 * qi + 64 * h2
                    for j in range(2):
                        rows = slice(h2 * 64 + 32 * j, h2 * 64 + 32 * j + 32)
                        nc.vector.tensor_tensor(
                            ynT[rows, pair, q0 : q0 + 512],
                            pv_sbs[qi][rows, :],
                            dnr[r : r + 32, :],
                            MUL,
                        )

            # ---- attention block ----------------------------------------------
            def attn_block(pair, qh, inject, dma_feed=None, premasks=None):
                dma_feed = dma_feed or {}
                pvs = []
                for i in range(2):
                    pv = ps_pv.tile([P, 512], F32, tag="pv", name=f"pv{i}")
                    nc.vector.memset(pv[:], 0.0)
                    pvs.append(pv)
                dn = ps_dn.tile([P, 512], F32, tag="dn", name="dn")

                def pv_dn_mms(kt, ptm2):
                    for qi in range(2):
                        for h2 in range(2):
                            nc.tensor.matmul(
                                pvs[qi][h2 * DOUT : (h2 + 1) * DOUT, :],
                                vsb4[:, kt, pair * 2 + h2, :],
                                ptm2[h2][:, qi * 512 : (qi + 1) * 512],
                                start=False,
                                stop=(kt == NKT - 1),
                                tile_position=(0, h2 * DOUT),
                                skip_group_check=True,
                            )
                    for qi in range(2):
                        for h2 in range(2):
                            row = 32 * qi + 64 * h2
                            nc.tensor.matmul(
                                dn[row : row + 32, :],
                                ones32_sb[:],
                                ptm2[h2][:, qi * 512 : (qi + 1) * 512],
                                start=False,
                                stop=(kt == NKT - 1),
                                tile_position=(0, row),
                                skip_group_check=True,
                            )

                masks = dict(premasks) if premasks else {}
                if premasks is None:
                    for k in range(3):
                        masks[k] = mask_dma(k, qh)
                ahead = 5 if premasks else 3

                ptms = {}
                for kt in range(NKT):
                    for th in dma_feed.get(kt, ()):
                        th()
                    # previous block's epilogue, spread over early kts
                    # (qi0 first so phaseC pieces over low q unblock sooner)
                    if pending and kt == 1:
                        flush_qi(pending[0], 0)
                    if pending and kt == 2:
                        flush_qi(pending[0], 1)
                    if kt == 2:
                        # after the old denominators' last read, before the
                        # first denominator matmul of this block
                        nc.vector.memset(dn[:], 0.0)
                    # trailing PV/dn: ready two tiles ago, fills the window
                    # while ScalarE still reads the previous S slots
                    if kt >= 2:
                        pv_dn_mms(kt - 2, ptms.pop(kt - 2))
                    if kt + ahead < NKT and (kt + ahead) not in masks:
                        masks[kt + ahead] = mask_dma(kt + ahead, qh)
                    sps = []
                    for h2 in range(2):
                        sp = ps_s.tile([P, 1024], F32, tag="s", name=f"sps{h2}")
                        sps.append(sp)
                    for qi in range(2):
                        for h2 in range(2):
                            q0 = (qh * 2 + qi) * 512
                            nc.tensor.matmul(
                                sps[h2][:, qi * 512 : (qi + 1) * 512],
                                khT[h2 * DOUT : (h2 + 1) * DOUT, pair, kt * P : (kt + 1) * P],
                                qhT[h2 * DOUT : (h2 + 1) * DOUT, pair, q0 : q0 + 512],
                                start=True,
                                stop=True,
                            )
                    mt = masks.pop(kt)
                    pts = []
                    for h2 in range(2):
                        pt = ptp.tile([P, 1024], BF, tag=f"pt{h2}", name="pt", bufs=2)
                        nc.scalar.activation(pt[:], sps[h2][:], EXP, scale=0.125)
                        pts.append(pt)
                    ptm2 = []
                    for h2 in range(2):
                        ptm = ptp.tile(
                            [P, 1024], BF, tag=f"ptm{h2}", name="ptm", bufs=3
                        )
                        nc.vector.tensor_mul(ptm[:], pts[h2][:], mt[:])
                        ptm2.append(ptm)
                    ptms[kt] = ptm2
                    # injected proj/output-projection tiles go last: they are
                    # lower scheduler priority than this kt's critical chain
                    for th in inject.get(kt, ()):
                        th()

                if pending:
                    pending.pop(0)
                # trailing tail: last two kt tiles
                for kt in (NKT - 2, NKT - 1):
                    pv_dn_mms(kt, ptms.pop(kt))
                # drain PV accumulators; in-place reciprocal of the denominators
                pv_sbs = []
                for qi in range(2):
                    pv_sb = epi.tile([P, 512], F32, tag="pvsb", name="pv_sb", bufs=4)
                    nc.vector.tensor_copy(pv_sb[:], pvs[qi][:])
                    pv_sbs.append(pv_sb)
                nc.vector.reciprocal_approx_fast(dn[:], dn[:])
                pending.append((pair, qh, pv_sbs, dn))

            # ---- prefix: minimal projections to unblock block A ---------------
            qk_tile("q", 0, 0, ps_s, "s")
            qk_tile("q", 0, 1, ps_s, "s")
            qk_tile("k", 0, 0, ps_pv, "pv")
            v_tile(0, ps_pv, "pv")
            v_tile(1, ps_dn, "dn")

            # ---- block A: (pair0, qh0); inject v st2-15, k-p0 qt1-3, q-p0 qt2-3
            inj_a = {kt: [] for kt in range(NKT)}
            for kt in range(1, 15):  # v st2..15 at kt 1..14 (kt0 stays clean)
                st = kt + 1
                inj_a[kt].append(lambda st=st: v_tile(st, ps_aux, "aux"))
            inj_a[1].append(lambda: qk_tile("k", 0, 1, ps_aux, "aux"))
            inj_a[3].append(lambda: qk_tile("k", 0, 2, ps_aux, "aux"))
            inj_a[7].append(lambda: qk_tile("k", 0, 3, ps_aux, "aux"))
            inj_a[9].append(lambda: qk_tile("q", 0, 2, ps_aux, "aux"))
            inj_a[11].append(lambda: qk_tile("q", 0, 3, ps_aux, "aux"))
            attn_block(0, 0, inj_a, dma_feed=dma_feed_a, premasks=premasks_a)

            # ---- block B: (pair0, qh1); inject first half of pair-1 q/k proj --
            inj_b = {
                1: [lambda: qk_tile("q", 1, 0, ps_aux, "aux")],
                4: [lambda: qk_tile("q", 1, 1, ps_aux, "aux")],
                7: [lambda: qk_tile("k", 1, 0, ps_aux, "aux")],
                10: [lambda: qk_tile("k", 1, 1, ps_aux, "aux")],
            }
            attn_block(0, 1, inj_b)

            # ---- block C: (pair1, qh0); inject rest of pair-1 proj ------------
            inj_c = {
                1: [lambda: qk_tile("k", 1, 2, ps_aux, "aux")],
                5: [lambda: qk_tile("k", 1, 3, ps_aux, "aux")],
                8: [lambda: qk_tile("q", 1, 2, ps_aux, "aux")],
                10: [lambda: qk_tile("q", 1, 3, ps_aux, "aux")],
            }
            attn_block(1, 0, inj_c)

            # ---- block D: (pair1, qh1); inject phaseC st0-7 -------------------
            # C's epilogue flushes at kt1 (qi0: q<512) and kt2 (qi1), so pieces
            # over q[0,512) can start at kt3 and the rest from kt4.
            pieces = [(st, nt) for st in range(8) for nt in range(2)]
            inj_d = {}
            for i, kt in enumerate(range(3, 16)):
                st, ntile = pieces[i]
                inj_d[kt] = [
                    lambda st=st, ntile=ntile: c_piece(st, ntile, ps_aux, "aux", "vector")
                ]
            rest = pieces[13:]  # (6,1),(7,0),(7,1) roll into the tail
            attn_block(1, 1, inj_d)

            # ---- tail: flush D's epilogue, then phaseC st8-15 ------------------
            ent = pending.pop(0)
            flush_qi(ent, 0)
            tail_pools = [(ps_aux, "aux"), (ps_pv, "pv"), (ps_s, "s")]
            tp_i = 0

            def tail_piece(st, ntile):
                nonlocal_i = tail_state[0]
                pool, tag = tail_pools[nonlocal_i % 3]
                eng = "scalar" if nonlocal_i % 2 == 0 else "vector"
                c_piece(st, ntile, pool, tag, eng)
                tail_state[0] = nonlocal_i + 1

            tail_state = [0]
            for st, ntile in rest:
                tail_piece(st, ntile)
            for st in (8, 9, 10, 11):
                for ntile in range(2):
                    tail_piece(st, ntile)
                if st == 8:
                    flush_qi(ent, 1)
            for st in (12, 13, 14, 15):
                for ntile in range(2):
                    tail_piece(st, ntile)

    nc.finalize()
    return nc


def make_in_maps(query, key, value, mask, Wq, bq, Wk, bk, Wv, bv, Wp, bp):
    """Shard + pre-layout the full inputs into 8 per-core input dicts."""
    in_maps = []
    for c in range(8):
        n = c // 4
        h0 = HPC * (c % 4)
        hs = slice(h0, h0 + HPC)

        def t_bf(x):  # [SEQ, DIN] -> contiguous [DIN, SEQ] bf16
            return np.ascontiguousarray(x.T).astype(BF_NP)

        # (H', DIN, DOUT) -> (DIN, H'*DOUT), head-major columns
        def w_bf(W):
            return np.ascontiguousarray(
                W[hs].transpose(1, 0, 2).reshape(DIN, HPC * DOUT)
            ).astype(BF_NP)

        # per-pair per-partition bias: [128, 2], col p = concat of heads (2p, 2p+1)
        def b_pair(b):
            return np.ascontiguousarray(b[hs].reshape(NPAIR, P).T).astype(np.float32)

        in_maps.append(
            {
                "xqT": t_bf(query[n]),
                "xkT": t_bf(key[n]),
                "xvT": t_bf(value[n]),
                "maskT": np.ascontiguousarray((~mask[n]).T).astype(BF_NP),
                "wq": w_bf(Wq),
                "wk": w_bf(Wk),
                "wv": w_bf(Wv),
                "wp": np.ascontiguousarray(
                    Wp[h0 * DOUT : (h0 + HPC) * DOUT, :]
                ).astype(BF_NP),
                "bqp": b_pair(bq),
                "bkp": b_pair(bk),
                "bvr": np.ascontiguousarray(
                    np.tile(bv[hs].reshape(1, HPC * DOUT), (P, 1))
                ).astype(np.float32),
            }
        )
    return in_maps


def kernel(**inputs):
    global _NC_CACHE
    from concourse.bass_utils import run_bass_kernel_spmd

    if _NC_CACHE is None:
        _NC_CACHE = build_bass()
    nc = _NC_CACHE

    in_maps = make_in_maps(**inputs)
    res = run_bass_kernel_spmd(nc, in_maps, core_ids=list(range(8))).results
    parts = [res[c]["out"].astype(np.float32) for c in range(8)]
    bp = inputs["bp"]
    out = np.stack(
        [
            parts[0] + parts[1] + parts[2] + parts[3] + bp[None, :],
            parts[4] + parts[5] + parts[6] + parts[7] + bp[None, :],
        ]
    )
    return out.astype(np.float32)
